# revision 1
# baseline (speedup 1.0000x reference)
"""Trainium2 Bass kernel for nn_LR_23029614641373 (embedding_lookup).

out[i] = [1-p, p],  p = sigmoid(w[u_i] + w[N_USERS + m_i] + b)
x: [B, 2] int (u, m), W: [1, 9923] f32, b: [1] f32, B = 4194304.

Strategy: pure data parallel over 8 NeuronCores (batch sharded), the
9923-entry table replicated into all 128 SBUF partitions on each core.
The per-element table lookups run on GPSIMD via InstAPGather: each Q7
core serves its 16 partitions with one gather stream, so one
instruction resolves 8 independent index streams (one per 16-partition
group).  The gathered stream for a group comes back replicated across
that group's 16 partitions; a block-diagonal SBUF->SBUF DMA compacts it
so every partition holds a distinct 1/16 slice, after which the
pair-add (w_u + w_m), sigmoid (+bias fused into ACT), and the
[1-p, p] construction (sigmoid with scale=-1, bias=-b) all run on
dense 128-partition tiles.  The output DMA unpermutes the gather
stream order back to row order (128-byte contiguous runs in DRAM).
"""

import numpy as np

N_USERS = 6040
N_MOVIES = 3883
TABLE = N_USERS + N_MOVIES  # 9923
B_TOTAL = 4194304
N_CORES = 8
ROWS_PER_CORE = B_TOTAL // N_CORES  # 524288

# Tile geometry (per core)
TILE_ROWS = 32768          # rows per tile
JP = TILE_ROWS // 128      # rows per partition per tile = 256
S16 = 2 * JP               # int16 index slots per partition = 512
NI = TILE_ROWS // 4        # ap_gather num_idxs (per 16-partition group) = 8192
SL = NI // 16              # compact slice length per partition = 512
PR = SL // 2               # logits per partition per tile = 256

_compiled = None

# Set by test harnesses: TRACE=True requests an NTFF profile; the full
# BassKernelResults of the last run is stashed in LAST_RESULTS.
TRACE = False
LAST_RESULTS = None


def _build(rows_per_core):
    import concourse.bacc as bacc
    import concourse.tile as tile
    from concourse import mybir

    n_tiles = rows_per_core // TILE_ROWS
    assert rows_per_core % TILE_ROWS == 0

    nc = bacc.Bacc()
    f32 = mybir.dt.float32
    i32 = mybir.dt.int32
    i16 = mybir.dt.int16

    x_d = nc.dram_tensor("x", [rows_per_core, 2], i32, kind="ExternalInput")
    w_d = nc.dram_tensor("w", [1, TABLE], f32, kind="ExternalInput")
    b_d = nc.dram_tensor("b", [1, 1], f32, kind="ExternalInput")
    y_d = nc.dram_tensor("y", [rows_per_core, 2], f32, kind="ExternalOutput")

    # DRAM views
    # input: partition p holds rows [p*JP, (p+1)*JP) of each tile (contiguous)
    x_t = x_d.rearrange("(t p j) c -> t p j c", p=128, j=JP)
    # output: row within tile = g*16*JP + l*JP + q*16 + jj
    # (partition 16g+q writes rows for all l, jj; jj+c merge into one
    # contiguous 32-element run so each DMA side stays <= 3 dims)
    y_t = y_d.rearrange(
        "(t g l q jj) c -> t g q l (jj c)", g=8, l=16, q=16, jj=JP // 16
    )

    with tile.TileContext(nc) as tc:
        with (
            tc.tile_pool(name="const", bufs=1) as constp,
            tc.tile_pool(name="xp", bufs=3) as xp,
            tc.tile_pool(name="ip", bufs=2) as ipool,
            tc.tile_pool(name="vp", bufs=2) as vp,
            tc.tile_pool(name="cp", bufs=2) as cp,
            tc.tile_pool(name="lp", bufs=2) as lp,
            tc.tile_pool(name="fp", bufs=3) as fp,
        ):
            # Table replicated across all 128 partitions + bias scalars
            Wt = constp.tile([128, TABLE], f32)
            nc.sync.dma_start(out=Wt[:, :], in_=w_d[0:1, :].to_broadcast((128, TABLE)))
            Bb = constp.tile([128, 1], f32)
            nc.sync.dma_start(out=Bb[:, :], in_=b_d[0:1, :].to_broadcast((128, 1)))
            nB = constp.tile([128, 1], f32)
            nc.vector.tensor_scalar_mul(nB[:, :], Bb[:, :], -1.0)

            for t in range(n_tiles):
                # Load x tile: [128, JP, 2] int32
                X = xp.tile([128, JP, 2], i32)
                nc.sync.dma_start(out=X[:, :, :], in_=x_t[t])

                # Index prep: I[p, 2j+0] = u, I[p, 2j+1] = N_USERS + m (int16)
                I = ipool.tile([128, S16], i16)
                I3 = I.rearrange("p (j c) -> p j c", c=2)
                nc.vector.tensor_copy(I3[:, :, 0], X[:, :, 0])
                nc.vector.tensor_scalar_add(I3[:, :, 1], X[:, :, 1], N_USERS)

                # Gather: group g's stream i = s*16 + l reads I[16g+l, s]
                # -> V[16g+*, i] = w[I[16g + i%16, i//16]]
                V = vp.tile([128, NI], f32)
                nc.gpsimd.ap_gather(V[:, :], Wt[:, :], I[:, :], 128, TABLE, 1, NI)

                # Compact: C[16g+q, j] = V[16g+q, q*SL + j]
                C = cp.tile([128, SL], f32)
                Vg = V.rearrange("(g q) i -> g q i", q=16)
                Cg = C.rearrange("(g q) j -> g q j", q=16)
                for q in range(16):
                    nc.sync.dma_start(
                        out=Cg[:, q, :], in_=Vg[:, q, q * SL : (q + 1) * SL]
                    )

                # Pair add: stream slot u512 = jj*32 + c*16 + l
                # A[p, jj*16 + l] = C[p, jj*32 + l] + C[p, jj*32 + 16 + l]
                A = lp.tile([128, PR], f32)
                C4 = C.rearrange("p (jj c l) -> p jj c l", c=2, l=16)
                A3 = A.rearrange("p (jj l) -> p jj l", l=16)
                nc.vector.tensor_add(A3[:, :, :], C4[:, :, 0, :], C4[:, :, 1, :])

                # F free layout: l*32 + jj*2 + c  (so the out-DMA free dims merge)
                # F[p, l*32+jj*2+1] = sigmoid(A+b); ...+0] = sigmoid(-A-b)
                F = fp.tile([128, 2 * PR], f32)
                F4 = F.rearrange("p (l jj c) -> p jj l c", l=16, c=2)
                nc.scalar.activation(
                    F4[:, :, :, 1], A3[:, :, :],
                    mybir.ActivationFunctionType.Sigmoid,
                    bias=Bb[:, 0:1], scale=1.0,
                )
                nc.scalar.activation(
                    F4[:, :, :, 0], A3[:, :, :],
                    mybir.ActivationFunctionType.Sigmoid,
                    bias=nB[:, 0:1], scale=-1.0,
                )

                # Output: F[16g+q, l*32+jj*2+c] -> row g*4096 + l*256 + q*16 + jj
                # One DMA per group g keeps both sides at 3 dims.
                Fv = F.rearrange("(g q) (l r) -> g q l r", q=16, r=32)
                for g in range(8):
                    nc.sync.dma_start(out=y_t[t, g], in_=Fv[g])

    nc.compile()
    return nc


def _get_compiled():
    global _compiled
    if _compiled is None:
        _compiled = _build(ROWS_PER_CORE)
    return _compiled


def kernel(x, W, b):
    from concourse.bass_utils import run_bass_kernel_spmd

    x = np.asarray(x)
    orig_rows = x.shape[0]
    assert x.shape == (B_TOTAL, 2), x.shape
    x32 = np.ascontiguousarray(x.astype(np.int32, copy=False))
    w = np.ascontiguousarray(np.asarray(W, dtype=np.float32).reshape(1, TABLE))
    bb = np.ascontiguousarray(np.asarray(b, dtype=np.float32).reshape(1, 1))

    nc = _get_compiled()
    in_maps = [
        {
            "x": x32[k * ROWS_PER_CORE : (k + 1) * ROWS_PER_CORE],
            "w": w,
            "b": bb,
        }
        for k in range(N_CORES)
    ]
    global LAST_RESULTS
    res = run_bass_kernel_spmd(nc, in_maps, list(range(N_CORES)), trace=TRACE)
    LAST_RESULTS = res
    out = np.concatenate([res.results[k]["y"] for k in range(N_CORES)], axis=0)
    assert out.shape == (orig_rows, 2)
    return out



# revision 2
# speedup vs baseline: 1.1874x; 1.1874x over previous
"""Trainium2 Bass kernel for nn_LR_23029614641373 (embedding_lookup).

out[i] = [1-p, p],  p = sigmoid(w[u_i] + w[N_USERS + m_i] + b)
x: [B, 2] int (u, m), W: [1, 9923] f32, b: [1] f32, B = 4194304.

Strategy: pure data parallel over 8 NeuronCores (batch sharded), the
9923-entry table replicated into all 128 SBUF partitions on each core.
The lookups run on GPSIMD via InstAPGather (8 independent streams, one
per 16-partition group).  Unlike the previous revision, the gathered
stream is kept *replicated* across each group's 16 partitions: the
pair-add and the two sigmoids are computed redundantly on the (idle)
Vector/Scalar engines, and the output DMA simply reads partition 16g
for group g — one 32 KB contiguous descriptor per group per tile.
This removes the SBUF->SBUF compaction DMAs and the 128-byte output
descriptors that made the old kernel descriptor-generation bound
(41.5k descriptors ~= 4.2 ms of serialized HWDGE work; now ~2.2k).

Layout per tile of 32768 rows (per core):
  input   partition p <- DRAM rows [t*32768 + p*256, +256)   (2 KB desc)
  indices partition 16g+q: u at I[:, 0:256], m+6040 at I[:, 256:512)
  stream  group g slot i -> row (i%16)*256 + i//16, u-half then m-half
  V[p, i] = w[stream_g(i)] replicated over the group's partitions
  A = V[:, :4096] + V[:, 4096:]               (logit, stream order)
  F[p, 2*(q*256+d)+c] = sigmoid(+/-(A[p, d*16+q]+b))  (row order)
  output  y rows of group g <- F[16g, :]       (8 x 32 KB descs)
"""

import numpy as np

N_USERS = 6040
N_MOVIES = 3883
TABLE = N_USERS + N_MOVIES  # 9923
B_TOTAL = 4194304
N_CORES = 8
ROWS_PER_CORE = B_TOTAL // N_CORES  # 524288

# Tile geometry (per core)
TILE_ROWS = 32768           # rows per tile
RP = TILE_ROWS // 128       # rows per partition per tile = 256
GR = TILE_ROWS // 8         # rows per group per tile = 4096
NI = 2 * GR                 # ap_gather num_idxs per group = 8192

_compiled = None

# Set by test harnesses: TRACE=True requests an NTFF profile; the full
# BassKernelResults of the last run is stashed in LAST_RESULTS.
TRACE = False
LAST_RESULTS = None


def _build(rows_per_core):
    import concourse.bacc as bacc
    import concourse.tile as tile
    from concourse import mybir

    n_tiles = rows_per_core // TILE_ROWS
    assert rows_per_core % TILE_ROWS == 0

    nc = bacc.Bacc()
    f32 = mybir.dt.float32
    i32 = mybir.dt.int32
    i16 = mybir.dt.int16

    x_d = nc.dram_tensor("x", [rows_per_core, 2], i32, kind="ExternalInput")
    w_d = nc.dram_tensor("w", [1, TABLE], f32, kind="ExternalInput")
    b_d = nc.dram_tensor("b", [1, 1], f32, kind="ExternalInput")
    y_d = nc.dram_tensor("y", [rows_per_core, 2], f32, kind="ExternalOutput")

    # DRAM views
    # input: partition p holds rows [t*TILE + p*RP, +RP) (contiguous 2 KB)
    x_t = x_d.rearrange("(t p r) c -> t p (r c)", p=128, r=RP)
    # output: group g of tile t covers rows [t*TILE + g*GR, +GR)
    y_t = y_d.rearrange("(t g r) c -> t g (r c)", g=8, r=GR)

    with tile.TileContext(nc) as tc:
        with (
            tc.tile_pool(name="const", bufs=1) as constp,
            tc.tile_pool(name="xp", bufs=2) as xp,
            tc.tile_pool(name="ip", bufs=2) as ipool,
            tc.tile_pool(name="vp", bufs=2) as vp,
            tc.tile_pool(name="ap", bufs=1) as apool,
            tc.tile_pool(name="fp", bufs=2) as fp,
        ):
            # Table replicated across all 128 partitions + bias scalars
            Wt = constp.tile([128, TABLE], f32)
            nc.sync.dma_start(out=Wt[:, :], in_=w_d[0:1, :].to_broadcast((128, TABLE)))
            Bb = constp.tile([128, 1], f32)
            nc.sync.dma_start(out=Bb[:, :], in_=b_d[0:1, :].to_broadcast((128, 1)))
            nB = constp.tile([128, 1], f32)
            nc.vector.tensor_scalar_mul(nB[:, :], Bb[:, :], -1.0)

            for t in range(n_tiles):
                # Load x tile: [128, RP, 2] int32 (one 2 KB desc / partition)
                X = xp.tile([128, RP, 2], i32)
                nc.sync.dma_start(out=X[:, :, :], in_=x_t[t])

                # Index prep: I[p, d] = u_d, I[p, RP+d] = N_USERS + m_d
                I = ipool.tile([128, 2 * RP], i16)
                nc.vector.tensor_copy(I[:, 0:RP], X[:, :, 0])
                nc.vector.tensor_scalar_add(I[:, RP : 2 * RP], X[:, :, 1], N_USERS)

                # Gather: group g stream slot i reads I[16g + i%16, i//16]
                # -> V[p, d*16 + q] = w[u of row q*RP+d]      (i < GR)
                #    V[p, GR + d*16 + q] = w[m' of row q*RP+d] (i >= GR)
                # replicated across the 16 partitions of each group.
                V = vp.tile([128, NI], f32)
                nc.gpsimd.ap_gather(V[:, :], Wt[:, :], I[:, :], 128, TABLE, 1, NI)

                # Pair add in stream order: A[p, j] = logit of row
                # (j%16)*RP + j//16 of group p//16.
                A = apool.tile([128, GR], f32)
                nc.vector.tensor_add(A[:, :], V[:, 0:GR], V[:, GR:NI])

                # Sigmoid + row-order transpose: F[p, 2*(q*RP+d)+c].
                # in-AP A[p, d*16+q] viewed as [p, q, d] (strides 1, 16).
                F = fp.tile([128, 2 * GR], f32)
                F4 = F.rearrange("p (q d c) -> p q d c", q=16, d=RP, c=2)
                A3 = A.rearrange("p (d q) -> p q d", q=16)
                nc.scalar.activation(
                    F4[:, :, :, 1], A3[:, :, :],
                    mybir.ActivationFunctionType.Sigmoid,
                    bias=Bb[:, 0:1], scale=1.0,
                )
                nc.scalar.activation(
                    F4[:, :, :, 0], A3[:, :, :],
                    mybir.ActivationFunctionType.Sigmoid,
                    bias=nB[:, 0:1], scale=-1.0,
                )

                # Output: group g rows from partition 16g only — 8 descs
                # of 32 KB.  Issued on the scalar HWDGE ring so input
                # (sync ring) and output descriptor generation overlap.
                Fg = F.rearrange("(g q) f -> q g f", q=16)
                nc.scalar.dma_start(out=y_t[t], in_=Fg[0])

    nc.compile()
    return nc


def _get_compiled():
    global _compiled
    if _compiled is None:
        _compiled = _build(ROWS_PER_CORE)
    return _compiled


def kernel(x, W, b):
    from concourse.bass_utils import run_bass_kernel_spmd

    x = np.asarray(x)
    orig_rows = x.shape[0]
    assert x.shape == (B_TOTAL, 2), x.shape
    x32 = np.ascontiguousarray(x.astype(np.int32, copy=False))
    w = np.ascontiguousarray(np.asarray(W, dtype=np.float32).reshape(1, TABLE))
    bb = np.ascontiguousarray(np.asarray(b, dtype=np.float32).reshape(1, 1))

    nc = _get_compiled()
    in_maps = [
        {
            "x": x32[k * ROWS_PER_CORE : (k + 1) * ROWS_PER_CORE],
            "w": w,
            "b": bb,
        }
        for k in range(N_CORES)
    ]
    global LAST_RESULTS
    res = run_bass_kernel_spmd(nc, in_maps, list(range(N_CORES)), trace=TRACE)
    LAST_RESULTS = res
    out = np.concatenate([res.results[k]["y"] for k in range(N_CORES)], axis=0)
    assert out.shape == (orig_rows, 2)
    return out


# revision 3
# speedup vs baseline: 1.1963x; 1.0075x over previous
"""Trainium2 Bass kernel for nn_LR_23029614641373 (embedding_lookup).

out[i] = [1-p, p],  p = sigmoid(w[u_i] + w[N_USERS + m_i] + b)
x: [B, 2] int (u, m), W: [1, 9923] f32, b: [1] f32, B = 4194304.

Strategy: pure data parallel over 8 NeuronCores (batch sharded), the
9923-entry table replicated into all 128 SBUF partitions on each core.
The lookups run on GPSIMD via InstAPGather (8 independent streams, one
per 16-partition group).  ap_gather is the binding constraint of this
whole problem: ~27.3 ns per index per Q7 stream (hardware-measured;
RD_CMD round-trip latency with Cayman ReadOverlap=0 — independent of
num_idxs, index values, and table size), i.e. ~3.4 ns per lookup per
NeuronCore with all 8 streams busy -> 2*524288 lookups ~= 3.58 ms.
Every other on-chip lookup mechanism is worse (indirect DMA: 1 index
per slowest AP dim; dma_gather: 256B rows; DVE/ACT: no data-dependent
addressing; PE one-hot: PSUM's 128-partition batch limit).

So the kernel's job is to keep everything else strictly inside the
gather shadow.  The gathered stream is kept *replicated* across each
group's 16 partitions: the pair-add and the two sigmoids are computed
redundantly on the (otherwise idle) Vector/Scalar engines, and the
output DMA simply reads partition 16g for group g — one 32 KB
contiguous descriptor per group per tile.  This removes the SBUF->SBUF
compaction DMAs and the 128-byte output descriptors of the previous
revision (41.5k descriptors -> ~2.2k; HWDGE generates ~1 descriptor
per ~100 ns, serialized per ring).  Measured: 3.685 ms vs 4.376 ms for
the previous revision in the same environment; inter-gather dead time
is ~0.4 us per tile.

Layout per tile of 32768 rows (per core):
  input   partition p <- DRAM rows [t*32768 + p*256, +256)   (2 KB desc)
  indices partition 16g+q: u at I[:, 0:256], m+6040 at I[:, 256:512)
  stream  group g slot i -> row (i%16)*256 + i//16, u-half then m-half
  V[p, i] = w[stream_g(i)] replicated over the group's partitions
  A = V[:, :4096] + V[:, 4096:]               (logit, stream order)
  F[p, 2*(q*256+d)+c] = sigmoid(+/-(A[p, d*16+q]+b))  (row order)
  output  y rows of group g <- F[16g, :]       (8 x 32 KB descs)
"""

import numpy as np

N_USERS = 6040
N_MOVIES = 3883
TABLE = N_USERS + N_MOVIES  # 9923
B_TOTAL = 4194304
N_CORES = 8
ROWS_PER_CORE = B_TOTAL // N_CORES  # 524288

# Tile geometry (per core)
TILE_ROWS = 32768           # rows per tile
RP = TILE_ROWS // 128       # rows per partition per tile = 256
GR = TILE_ROWS // 8         # rows per group per tile = 4096
NI = 2 * GR                 # ap_gather num_idxs per group = 8192

_compiled = None

# Set by test harnesses: TRACE=True requests an NTFF profile; the full
# BassKernelResults of the last run is stashed in LAST_RESULTS.
TRACE = False
LAST_RESULTS = None


def _build(rows_per_core):
    import concourse.bacc as bacc
    import concourse.tile as tile
    from concourse import mybir

    n_tiles = rows_per_core // TILE_ROWS
    assert rows_per_core % TILE_ROWS == 0

    nc = bacc.Bacc()
    f32 = mybir.dt.float32
    i32 = mybir.dt.int32
    i16 = mybir.dt.int16

    x_d = nc.dram_tensor("x", [rows_per_core, 2], i32, kind="ExternalInput")
    w_d = nc.dram_tensor("w", [1, TABLE], f32, kind="ExternalInput")
    b_d = nc.dram_tensor("b", [1, 1], f32, kind="ExternalInput")
    y_d = nc.dram_tensor("y", [rows_per_core, 2], f32, kind="ExternalOutput")

    # DRAM views
    # input: partition p holds rows [t*TILE + p*RP, +RP) (contiguous 2 KB)
    x_t = x_d.rearrange("(t p r) c -> t p (r c)", p=128, r=RP)
    # output: group g of tile t covers rows [t*TILE + g*GR, +GR)
    y_t = y_d.rearrange("(t g r) c -> t g (r c)", g=8, r=GR)

    with tile.TileContext(nc) as tc:
        with (
            tc.tile_pool(name="const", bufs=1) as constp,
            tc.tile_pool(name="xp", bufs=2) as xp,
            tc.tile_pool(name="ip", bufs=2) as ipool,
            tc.tile_pool(name="vp", bufs=2) as vp,
            tc.tile_pool(name="ap", bufs=1) as apool,
            tc.tile_pool(name="fp", bufs=2) as fp,
        ):
            # Table replicated across all 128 partitions + bias scalars
            Wt = constp.tile([128, TABLE], f32)
            nc.sync.dma_start(out=Wt[:, :], in_=w_d[0:1, :].to_broadcast((128, TABLE)))
            Bb = constp.tile([128, 1], f32)
            nc.sync.dma_start(out=Bb[:, :], in_=b_d[0:1, :].to_broadcast((128, 1)))
            nB = constp.tile([128, 1], f32)
            nc.vector.tensor_scalar_mul(nB[:, :], Bb[:, :], -1.0)

            for t in range(n_tiles):
                # Load x tile: [128, RP, 2] int32 (one 2 KB desc / partition)
                X = xp.tile([128, RP, 2], i32)
                nc.sync.dma_start(out=X[:, :, :], in_=x_t[t])

                # Index prep: I[p, d] = u_d, I[p, RP+d] = N_USERS + m_d
                I = ipool.tile([128, 2 * RP], i16)
                nc.vector.tensor_copy(I[:, 0:RP], X[:, :, 0])
                nc.vector.tensor_scalar_add(I[:, RP : 2 * RP], X[:, :, 1], N_USERS)

                # Gather: group g stream slot i reads I[16g + i%16, i//16]
                # -> V[p, d*16 + q] = w[u of row q*RP+d]      (i < GR)
                #    V[p, GR + d*16 + q] = w[m' of row q*RP+d] (i >= GR)
                # replicated across the 16 partitions of each group.
                V = vp.tile([128, NI], f32)
                nc.gpsimd.ap_gather(V[:, :], Wt[:, :], I[:, :], 128, TABLE, 1, NI)

                # Pair add in stream order: A[p, j] = logit of row
                # (j%16)*RP + j//16 of group p//16.
                A = apool.tile([128, GR], f32)
                nc.vector.tensor_add(A[:, :], V[:, 0:GR], V[:, GR:NI])

                # Sigmoid + row-order transpose: F[p, 2*(q*RP+d)+c].
                # in-AP A[p, d*16+q] viewed as [p, q, d] (strides 1, 16).
                F = fp.tile([128, 2 * GR], f32)
                F4 = F.rearrange("p (q d c) -> p q d c", q=16, d=RP, c=2)
                A3 = A.rearrange("p (d q) -> p q d", q=16)
                nc.scalar.activation(
                    F4[:, :, :, 1], A3[:, :, :],
                    mybir.ActivationFunctionType.Sigmoid,
                    bias=Bb[:, 0:1], scale=1.0,
                )
                nc.scalar.activation(
                    F4[:, :, :, 0], A3[:, :, :],
                    mybir.ActivationFunctionType.Sigmoid,
                    bias=nB[:, 0:1], scale=-1.0,
                )

                # Output: group g rows from partition 16g only — 8 descs
                # of 32 KB.  Issued on the scalar HWDGE ring so input
                # (sync ring) and output descriptor generation overlap.
                Fg = F.rearrange("(g q) f -> q g f", q=16)
                nc.scalar.dma_start(out=y_t[t], in_=Fg[0])

    nc.compile()
    return nc


def _get_compiled():
    global _compiled
    if _compiled is None:
        _compiled = _build(ROWS_PER_CORE)
    return _compiled


def kernel(x, W, b):
    from concourse.bass_utils import run_bass_kernel_spmd

    x = np.asarray(x)
    orig_rows = x.shape[0]
    assert x.shape == (B_TOTAL, 2), x.shape
    x32 = np.ascontiguousarray(x.astype(np.int32, copy=False))
    w = np.ascontiguousarray(np.asarray(W, dtype=np.float32).reshape(1, TABLE))
    bb = np.ascontiguousarray(np.asarray(b, dtype=np.float32).reshape(1, 1))

    nc = _get_compiled()
    in_maps = [
        {
            "x": x32[k * ROWS_PER_CORE : (k + 1) * ROWS_PER_CORE],
            "w": w,
            "b": bb,
        }
        for k in range(N_CORES)
    ]
    global LAST_RESULTS
    res = run_bass_kernel_spmd(nc, in_maps, list(range(N_CORES)), trace=TRACE)
    LAST_RESULTS = res
    out = np.concatenate([res.results[k]["y"] for k in range(N_CORES)], axis=0)
    assert out.shape == (orig_rows, 2)
    return out
